# revision 13
# baseline (speedup 1.0000x reference)
"""Trainium2 Bass kernel for nn_BiasedInterpretedFlockingModel (GNN message passing).

Sharding/strategy
-----------------
Host (numpy, inside kernel()):
  * Sort edges by destination node and compute per-edge x = pos[src] - pos[dst].
    (8-byte random gathers via DMA descriptors cost ~7ns/descriptor on SDMA --
    hopeless for 6.4M edges; the access pattern is pure edge_index preprocessing.)
  * Deal nodes round-robin by degree rank across 8 cores so every core gets an
    identical tile structure (pure SPMD: one program, per-core data, no
    collectives -- each core owns all edges of its own nodes).
  * Pack each core's edges into a dense [128, F] f32 image: 128-node tiles (one
    node per partition), K slots per tile (tile max degree, zero padded).  Zero
    slots contribute exactly 0 to every reduced channel.
  * deg / unmasked-count are exact functions of edge_index (+ float equality of
    node features), computed on host.

Device (per core, per chunk of equal-K tiles):
  ACT : q = Square(CA*x0);  u = q + CB;  s = Square(CE1*x0)
  DVE : r = recip_approx(u); pr = x1*r; ps = x1*s
        segmented reduces ([P,b,k] -> [P,b], axis=X):
        A=sum(x0)  B=sum(x1)  C=sum(pr)  D=sum(ps)  F=sum(s)
  epilogue on [128, T] per-node arrays: message sums -> y -> u -> (p0, p1).

Message algebra (validated against the reference in fp64/fp32):
  sum_m0 = C0M*(A - C)                        (m0 core is odd in x; 0 when x=0)
  sum_m1 = CF1*(A - CD1*B + D)
  sum_m2 = CH2*(A - WGE*F) + CB2*cntU         (w = (CG2 x0)^2 = WGE * s)
  sum_m3 = CL3*(B + CK3*A) + CL3*CJ3*cntU
  y = [sum_m2, sum_m3, sum_m0/max(deg,1), sum_m1/max(deg,1)]
"""

import sys

import numpy as np

sys.path.insert(0, "/opt/trn_rl_repo")

import concourse.bacc as bacc
import concourse.mybir as mybir
import concourse.tile as tile
from concourse import bass_utils

N_NODES = 100000
N_EDGES = 6400000
NCORES = 8
P = 128
NPC = (N_NODES + NCORES - 1) // NCORES          # nodes per core = 12500
T = (NPC + P - 1) // P                          # node tiles per core = 98
RANKS = T * P * NCORES                          # padded global rank slots
KQUANT = 4
MAX_CHUNK_COLS = 1664

F32 = mybir.dt.float32
AX = mybir.AxisListType
OP = mybir.AluOpType
AF = mybir.ActivationFunctionType

# model constants
CA = 0.07104663
CB = 1.536996
C0M = -0.028956918
CD1 = 0.8290067
CF1 = 0.025425926
CE1 = -0.021992652
CG2 = -0.083299406
CH2 = -0.024002103
CB2 = -0.22298379
CK3 = -0.16023761
CL3 = 0.025031794
CJ3 = 2.6200492
WGE = (CG2 * CG2) / (CE1 * CE1)
C15 = 0.15994334
C17 = 1.7044706
C16 = 0.16596459
C08 = 0.089175865
CU1 = -0.05459863
CU2 = 0.05392959
CU3 = 12.305774
CD3 = 63.129406
CP05 = 0.5268826
CP0A = -0.18549965
CGAM = 0.7328953
CP1A = -0.8037861
CP1B = 1.2175907


def _plan_chunks(k_per_tile):
    """Group consecutive equal-K tiles into chunks of <= MAX_CHUNK_COLS cols."""
    chunks = []  # (tile_start, ntiles, K)
    t = 0
    while t < T:
        k = int(k_per_tile[t])
        b = 1
        while (t + b < T and int(k_per_tile[t + b]) == k
               and (b + 1) * k <= MAX_CHUNK_COLS):
            b += 1
        chunks.append((t, b, k))
        t += b
    return chunks


def _build_nc(chunks, stage=99, loop_n=None):
    """Build the SPMD Bass/Tile program (same program for all 8 cores).

    stage (debug): 1=DMA+ACT only, 2=+DVE elementwise, 3=+reduces,
    4=+epilogue (full).
    loop_n: if set, wrap the whole body in a hardware For_i loop (for
    steady-state device timing via wall-clock deltas).
    """
    fcols = sum(b * k for (_, b, k) in chunks)
    nc = bacc.Bacc("TRN2", target_bir_lowering=False, debug=False,
                   num_devices=NCORES)

    edata = nc.dram_tensor("edata", [P, 2 * fcols], F32, kind="ExternalInput")
    ndata = nc.dram_tensor("ndata", [P, 3 * T], F32, kind="ExternalInput")
    out = nc.dram_tensor("out", [P, 2 * T], F32, kind="ExternalOutput")
    e_ap, n_ap, o_ap = edata.ap(), ndata.ap(), out.ap()

    v = nc.vector
    sc = nc.scalar

    from contextlib import nullcontext

    with tile.TileContext(nc) as tc:
        with (
            tc.tile_pool(name="io", bufs=3) as io_pool,
            tc.tile_pool(name="scr", bufs=2) as scr_pool,
            tc.tile_pool(name="stat", bufs=1) as stat_pool,
            tc.For_i(0, loop_n, 1) if loop_n is not None else nullcontext(),
        ):
            stats = stat_pool.tile([P, 5 * T], F32, tag="stats")
            nd = stat_pool.tile([P, 3 * T], F32, tag="nd")
            nc.sync.dma_start(nd[:], n_ap[:, :])
            cbb = stat_pool.tile([P, 1], F32, tag="cbb")
            v.memset(cbb[:], CB)

            v.memset(stats[:], 0.0)

            col = 0
            for (t0, b, k) in chunks:
                w = b * k
                buf = io_pool.tile([P, 2 * w], F32, tag="edata")
                nc.sync.dma_start(buf[:], e_ap[:, 2 * col:2 * col + 2 * w])
                x0 = buf[:, 0:w]
                x1 = buf[:, w:2 * w]

                q = scr_pool.tile([P, w], F32, tag="q")
                sc.activation(q[:], x0, AF.Square, scale=CA)
                u = scr_pool.tile([P, w], F32, tag="u")
                sc.activation(u[:], q[:], AF.Identity, bias=cbb[:])
                s = scr_pool.tile([P, w], F32, tag="s")
                sc.activation(s[:], x0, AF.Square, scale=CE1)

                if stage < 2:
                    col += w
                    continue
                r = scr_pool.tile([P, w], F32, tag="r")
                v.reciprocal_approx_fast(out=r[:], in_=u[:])

                pr = scr_pool.tile([P, w], F32, tag="pr")
                v.tensor_tensor(pr[:], x1, r[:], OP.mult)
                ps = scr_pool.tile([P, w], F32, tag="ps")
                v.tensor_tensor(ps[:], x1, s[:], OP.mult)

                if stage < 3:
                    col += w
                    continue
                for off, srcv in ((0, x0), (1, x1), (2, pr[:]), (3, ps[:]),
                                  (4, s[:])):
                    v.reduce_sum(
                        stats[:, off * T + t0: off * T + t0 + b],
                        srcv.rearrange("p (b k) -> p b k", b=b, k=k),
                        axis=AX.X,
                    )
                col += w

            # ---------------- epilogue on [P, T] ----------------
            A = stats[:, 0 * T:1 * T]
            B = stats[:, 1 * T:2 * T]
            C = stats[:, 2 * T:3 * T]
            D = stats[:, 3 * T:4 * T]
            F = stats[:, 4 * T:5 * T]
            invd = nd[:, 0 * T:1 * T]
            cb2c = nd[:, 1 * T:2 * T]     # CB2 * cntU
            cljc = nd[:, 2 * T:3 * T]     # CL3 * CJ3 * cntU

            ep = stat_pool.tile([P, 14 * T], F32, tag="ep")

            def sl(i):
                return ep[:, i * T:(i + 1) * T]

            y0, y1, y2, y3 = sl(0), sl(1), sl(2), sl(3)
            z, u0p, u1p, u2p, u3p = sl(4), sl(5), sl(6), sl(7), sl(8)
            ta, tb, tcs = sl(9), sl(10), sl(11)
            p0s, p1s = sl(12), sl(13)

            def stt(out_, in0, scalar, in1, op0, op1):
                v.scalar_tensor_tensor(out_, in0, float(scalar), in1, op0, op1)

            # y0 = CH2*A - CH2*WGE*F + CB2*cntU
            stt(ta, F, -(CH2 * WGE), cb2c, OP.mult, OP.add)
            stt(y0, A, CH2, ta, OP.mult, OP.add)
            # y1 = CL3*B + CL3*CK3*A + CL3*CJ3*cntU
            stt(ta, A, CL3 * CK3, cljc, OP.mult, OP.add)
            stt(y1, B, CL3, ta, OP.mult, OP.add)
            # y2 = C0M*(A - C)*invd
            v.tensor_tensor(ta, A, C, OP.subtract)
            stt(y2, ta, C0M, invd, OP.mult, OP.mult)
            # y3 = CF1*(A - CD1*B + D)*invd
            stt(ta, B, -CD1, D, OP.mult, OP.add)
            v.tensor_tensor(tb, ta, A, OP.add)
            stt(y3, tb, CF1, invd, OP.mult, OP.mult)

            # z = (C15*y2)^2
            sc.activation(z, y2, AF.Square, scale=C15)
            # u0p = (y0-y2) - (y3+z)/C17      [u0 = C16*u0p]
            v.tensor_tensor(ta, y3, z, OP.add)
            v.tensor_tensor(tb, y0, y2, OP.subtract)
            stt(u0p, ta, -1.0 / C17, tb, OP.mult, OP.add)
            # u1p = y1 - (C08^2/C15^2)*z*y3 + (y3-y2)    [u1 = CU1*u1p]
            v.tensor_tensor(ta, z, y3, OP.mult)
            stt(tb, ta, -(C08 * C08) / (C15 * C15), y1, OP.mult, OP.add)
            v.tensor_tensor(tcs, y3, y2, OP.subtract)
            v.tensor_tensor(u1p, tb, tcs, OP.add)
            # u2p = y3 + y0                   [u2 = CU2*u2p]
            v.tensor_tensor(u2p, y3, y0, OP.add)
            # u3p = y2/(y2^2 + CD3)           [u3 = CU3*u3p]
            v.tensor_scalar(ta, z, 1.0 / (C15 * C15), CD3, OP.mult, OP.add)
            v.reciprocal_approx_fast(out=tb, in_=ta)
            v.tensor_tensor(u3p, y2, tb, OP.mult)

            # p0 = ((C16/CP05*u0p + CU3*u3p - CU2*u2p)*CP0A - CU1*u1p - CU2*u2p)/CGAM
            v.tensor_scalar_mul(ta, u0p, C16 / CP05)
            stt(tb, u3p, CU3, ta, OP.mult, OP.add)
            stt(ta, u2p, -CU2, tb, OP.mult, OP.add)          # inner
            v.tensor_scalar_mul(tb, u1p, -CU1 / CGAM)
            stt(tcs, u2p, -CU2 / CGAM, tb, OP.mult, OP.add)
            stt(p0s, ta, CP0A / CGAM, tcs, OP.mult, OP.add)

            # p1 = CP1A*C16*u0p - CU1*u1p + CP1B*CU3*u3p + CU2*u2p
            v.tensor_scalar_mul(tb, u2p, CU2)
            stt(tcs, u1p, -CU1, tb, OP.mult, OP.add)
            stt(tb, u3p, CP1B * CU3, tcs, OP.mult, OP.add)
            stt(p1s, u0p, CP1A * C16, tb, OP.mult, OP.add)

            nc.sync.dma_start(o_ap[:, 0:T], p0s)
            nc.sync.dma_start(o_ap[:, T:2 * T], p1s)

    nc.compile()
    return nc


def _preprocess(pos, vel, edge_index):
    pos = np.ascontiguousarray(np.asarray(pos, dtype=np.float32))
    vel = np.ascontiguousarray(np.asarray(vel, dtype=np.float32))
    ei = np.asarray(edge_index)
    src = np.ascontiguousarray(ei[0]).astype(np.int64, copy=False)
    dst = np.ascontiguousarray(ei[1]).astype(np.int64, copy=False)

    deg = np.bincount(dst, minlength=N_NODES)
    meq = ((pos[src, 0] == pos[dst, 0]) & (pos[src, 1] == pos[dst, 1])
           & (vel[src, 0] == vel[dst, 0]) & (vel[src, 1] == vel[dst, 1]))
    nmask = np.bincount(dst[meq], minlength=N_NODES)
    cntU = (deg - nmask).astype(np.float32)
    degf = deg.astype(np.float32)

    # rank nodes by degree (desc); rank r -> core r%8, slot r//8
    nodeorder = np.argsort(-deg, kind="stable")          # rank -> node
    rank = np.empty(N_NODES, dtype=np.int64)
    rank[nodeorder] = np.arange(N_NODES)

    # per-tile K (ranks [t*1024, (t+1)*1024) form tile t on all cores)
    k_per_tile = np.empty(T, dtype=np.int64)
    for t in range(T):
        d = int(deg[nodeorder[t * P * NCORES]])
        k_per_tile[t] = max(KQUANT, -(-d // KQUANT) * KQUANT)
    chunks = _plan_chunks(k_per_tile)
    fcols = sum(b * k for (_, b, k) in chunks)

    # per-tile column bases for x0/x1 inside the [P, 2*fcols] image
    x0base = np.zeros(T, dtype=np.int64)
    x1base = np.zeros(T, dtype=np.int64)
    col = 0
    for (t0, b, k) in chunks:
        w = b * k
        for i in range(b):
            x0base[t0 + i] = 2 * col + i * k
            x1base[t0 + i] = 2 * col + w + i * k
        col += w

    # per-edge placement (edges sorted by dst)
    order = np.argsort(dst, kind="stable")
    dsts = dst[order]
    srcs = src[order]
    starts = np.concatenate(([0], np.cumsum(deg)[:-1]))
    j = np.arange(N_EDGES, dtype=np.int64) - starts[dsts]

    x = pos[dsts] - pos[srcs]                            # [E, 2] f32 (= d)
    rk = rank[dsts]
    core = rk % NCORES
    slot = rk // NCORES
    tt = slot // P
    pp = slot % P

    edata = np.zeros((NCORES, P, 2 * fcols), dtype=np.float32)
    c0 = x0base[tt] + j
    edata[core, pp, c0] = x[:, 0]
    edata[core, pp, x1base[tt] + j] = x[:, 1]

    # ndata: invd | CB2*cntU | CL3*CJ3*cntU  at [p, block*T + t]
    ndata = np.zeros((NCORES, P, 3 * T), dtype=np.float32)
    r_all = np.arange(RANKS, dtype=np.int64)
    n_all = np.full(RANKS, -1, dtype=np.int64)
    n_all[:N_NODES] = nodeorder
    corea = r_all % NCORES
    slota = r_all // NCORES
    ta_ = slota // P
    pa = slota % P
    valid = n_all >= 0
    iv = np.ones(RANKS, dtype=np.float32)
    cb2 = np.zeros(RANKS, dtype=np.float32)
    clj = np.zeros(RANKS, dtype=np.float32)
    iv[valid] = 1.0 / np.maximum(degf[n_all[valid]], 1.0)
    cb2[valid] = np.float32(CB2) * cntU[n_all[valid]]
    clj[valid] = np.float32(CL3 * CJ3) * cntU[n_all[valid]]
    ndata[corea, pa, ta_] = iv
    ndata[corea, pa, T + ta_] = cb2
    ndata[corea, pa, 2 * T + ta_] = clj

    meta = dict(chunks=tuple(chunks), corea=corea[valid], pa=pa[valid],
                ta=ta_[valid], nodes=n_all[valid])
    return edata, ndata, meta


_NC_CACHE = {}


def kernel(pos, vel, edge_index):
    edata, ndata, meta = _preprocess(pos, vel, edge_index)
    key = meta["chunks"]
    nc = _NC_CACHE.get(key)
    if nc is None:
        nc = _build_nc(key)
        _NC_CACHE[key] = nc

    in_maps = [{"edata": edata[c], "ndata": ndata[c]} for c in range(NCORES)]
    res = bass_utils.run_bass_kernel_spmd(nc, in_maps, core_ids=list(range(NCORES)))

    outf = np.empty((N_NODES, 2), dtype=np.float32)
    for c in range(NCORES):
        o = res.results[c]["out"]
        m = meta["corea"] == c
        outf[meta["nodes"][m], 0] = o[meta["pa"][m], meta["ta"][m]]
        outf[meta["nodes"][m], 1] = o[meta["pa"][m], T + meta["ta"][m]]
    return outf
